# revision 1
# baseline (speedup 1.0000x reference)
"""MultiHeadAttention on 8 TRN2 NeuronCores.

Shapes (hardcoded): x [4, 2048, 1024], w_qkv [1024, 3072], b_qkv [3072],
w_o [1024, 1024], b_o [1024]; H=16 heads, head_dim=64, scale 1/8.

Sharding: core c -> batch c//2, head-group c%2 (8 heads each).
Each core computes its 8 heads' attention values and a partial o-proj
([2048, 1024] f32); host sums the two partials per batch, adds b_o and
the constant row bv @ w_o (v-bias folds out of attention since softmax
rows sum to 1).

V2 single-phase schedule (all matmuls bf16): projections and o-proj
matmuls are interleaved into the attention steps so the PE array never
idles; the softmax normalization chain of each (pair, query-block) step
is deferred into the next step so the PE does not stall on the DVE
reciprocal; bc_e/bc_o broadcast matmuls stack into one PSUM bank at
partition bases 0/64.
"""

import os
import sys
import types

sys.path.insert(0, "/opt/trn_rl_repo")

import numpy as np
import ml_dtypes
from contextlib import ExitStack

import concourse.bass as bass  # noqa: F401
import concourse.tile as tile
from concourse import bacc, mybir
from concourse.bass_utils import run_bass_kernel_spmd

BF16 = mybir.dt.bfloat16
F32 = mybir.dt.float32
NBF = ml_dtypes.bfloat16

N_CORES = 8
B, S, D, E = 4, 2048, 1024, 1024
H, HD = 16, 64
NP = 4    # head pairs per core
NQB = 4   # query blocks of 512
NKC = 16  # key/seq chunks of 128
NIC = 8   # input-dim chunks of 128

TRACE = os.environ.get("KERNEL_TRACE", "") == "1"
LAST_EXEC_NS = None

if TRACE:
    _hook = [None]
    _ah = types.ModuleType("antenv.axon_hooks")
    _ah.set_axon_ntff_profile_hook = lambda h: _hook.__setitem__(0, h)
    _ah.get_axon_ntff_profile_hook = lambda: _hook[0]
    sys.modules["antenv.axon_hooks"] = _ah
    import antenv
    antenv.axon_hooks = _ah
    from trn_agent_boot.trn_boot import _ntff_profile_via_ctypes
    _ah.set_axon_ntff_profile_hook(
        _ntff_profile_via_ctypes("/opt/axon/libaxon_pjrt.so"))

_nc_cache = [None]


def _build():
    nc = bacc.Bacc("TRN2", target_bir_lowering=False, debug=False,
                   num_devices=N_CORES)
    xT_ap = nc.dram_tensor("xT", [NIC, 128, S], BF16, kind="ExternalInput").ap()
    wq_ap = nc.dram_tensor("wq", [NIC, 128, 512], BF16, kind="ExternalInput").ap()
    wk_ap = nc.dram_tensor("wk", [NIC, 128, 512], BF16, kind="ExternalInput").ap()
    wv_ap = nc.dram_tensor("wv", [NIC, 128, 512], BF16, kind="ExternalInput").ap()
    wo_ap = nc.dram_tensor("wo", [NP, 128, 1024], BF16, kind="ExternalInput").ap()
    bq_ap = nc.dram_tensor("bq", [128, NP], F32, kind="ExternalInput").ap()
    bk_ap = nc.dram_tensor("bk", [128, NP], F32, kind="ExternalInput").ap()
    out_ap = nc.dram_tensor("out", [NKC, 128, 1024], F32,
                            kind="ExternalOutput").ap()

    with tile.TileContext(nc) as tc:
        with ExitStack() as ctx:
            sb = ctx.enter_context(tc.tile_pool(name="sb", bufs=1))
            xT_sb = sb.tile([128, NIC, S], BF16)
            wq_sb = sb.tile([128, NIC, 512], BF16)
            wk_sb = sb.tile([128, NIC, 512], BF16)
            wv_sb = sb.tile([128, NIC, 512], BF16)
            wo_sb = sb.tile([128, NP, 1024], BF16)
            bq_sb = sb.tile([128, NP], F32)
            bk_sb = sb.tile([128, NP], F32)
            v_aug = sb.tile([128, NKC, 8, 65], BF16)
            ones_col = sb.tile([128, 64], BF16)
            qT = [sb.tile([128, S], BF16, name=f"qT{p}") for p in range(NP)]
            kT = [sb.tile([128, S], BF16, name=f"kT{p}") for p in range(NP)]
            valsT = [sb.tile([128, S], BF16, name=f"valsT{p}")
                     for p in range(NP)]

            for ic in range(NIC):
                nc.sync.dma_start(out=xT_sb[:, ic, :], in_=xT_ap[ic])
                nc.sync.dma_start(out=wv_sb[:, ic, :], in_=wv_ap[ic])
            for ic in range(NIC):
                nc.sync.dma_start(out=wk_sb[:, ic, :], in_=wk_ap[ic])
                nc.sync.dma_start(out=wq_sb[:, ic, :], in_=wq_ap[ic])
            nc.sync.dma_start(out=bq_sb[:], in_=bq_ap[:])
            nc.sync.dma_start(out=bk_sb[:], in_=bk_ap[:])
            for p in range(NP):
                nc.sync.dma_start(out=wo_sb[:, p, :], in_=wo_ap[p])
            nc.gpsimd.memset(v_aug[:], 1.0)
            nc.gpsimd.memset(ones_col[:], 1.0)

            qkps = ctx.enter_context(
                tc.tile_pool(name="qkps", bufs=2, space="PSUM"))
            avps = ctx.enter_context(
                tc.tile_pool(name="avps", bufs=1, space="PSUM"))
            accs = ctx.enter_context(
                tc.tile_pool(name="accs", bufs=2, space="PSUM"))
            eps = ctx.enter_context(tc.tile_pool(name="eps", bufs=3))
            rbs = ctx.enter_context(tc.tile_pool(name="rbs", bufs=2))
            ost = ctx.enter_context(tc.tile_pool(name="ost", bufs=2))

            def vproj_group(kc):
                acc = accs.tile([128, 512], F32, name="acc")
                for ic in range(NIC):
                    nc.tensor.matmul(
                        acc[:], xT_sb[:, ic, kc * 128:(kc + 1) * 128],
                        wv_sb[:, ic, :],
                        start=(ic == 0), stop=(ic == NIC - 1))
                nc.vector.tensor_copy(v_aug[:, kc, :, 0:64], acc[:])

            def proj_group(which, p, qb):
                qcols = slice(qb * 512, (qb + 1) * 512)
                pcols = slice(p * 128, (p + 1) * 128)
                w_sb, b_sb, dst = ((wq_sb, bq_sb, qT) if which == "q"
                                   else (wk_sb, bk_sb, kT))
                acc = accs.tile([128, 512], F32, name="acc")
                for ic in range(NIC):
                    nc.tensor.matmul(
                        acc[:], w_sb[:, ic, pcols], xT_sb[:, ic, qcols],
                        start=(ic == 0), stop=(ic == NIC - 1))
                nc.vector.tensor_scalar_add(
                    dst[p][:, qcols], acc[:], b_sb[:, p:p + 1])

            def oproj_sc(sc):
                scols = slice(sc * 128, (sc + 1) * 128)
                stage = ost.tile([128, 1024], F32, name="ostage")
                for half in range(2):
                    hcols = slice(half * 512, (half + 1) * 512)
                    og = accs.tile([128, 512], F32, name="acc")
                    for p in range(NP):
                        nc.tensor.matmul(
                            og[:], valsT[p][:, scols], wo_sb[:, p, hcols],
                            start=(p == 0), stop=(p == NP - 1))
                    nc.vector.tensor_copy(stage[:, hcols], og[:])
                nc.sync.dma_start(out=out_ap[sc], in_=stage[:])

            pend_norm = [None]

            def make_norm(p, qb, av, recip):
                qcols = slice(qb * 512, (qb + 1) * 512)

                def norm():
                    bc = accs.tile([128, 512], F32, name="acc")
                    nc.tensor.matmul(
                        bc[0:64, :], ones_col[64:65, :],
                        recip[64:65, 0:512], start=True, stop=True)
                    nc.tensor.matmul(
                        bc[64:128, :], ones_col[64:65, :],
                        recip[64:65, 512:1024], start=True, stop=True)
                    bc_sb = rbs.tile([128, 1024], BF16, name="bcsb")
                    nc.vector.tensor_copy(bc_sb[0:64, 0:512], bc[0:64, :])
                    nc.vector.tensor_copy(bc_sb[0:64, 512:1024],
                                          bc[64:128, :])
                    nc.vector.tensor_mul(
                        valsT[p][0:64, qcols], av[0:64, 0:512],
                        bc_sb[0:64, 0:512])
                    nc.vector.tensor_mul(
                        valsT[p][64:128, qcols], av[0:64, 512:1024],
                        bc_sb[0:64, 512:1024])
                return norm

            def attention_step(p, qb, fillers):
                qcols = slice(qb * 512, (qb + 1) * 512)
                ets = {}

                def qk(kc):
                    kcols = slice(kc * 128, (kc + 1) * 128)
                    slot = qkps.tile([128, 1024], F32, name="qkslot")
                    nc.tensor.matmul(
                        slot[:, 0:512], kT[p][0:64, kcols],
                        qT[p][0:64, qcols], start=True, stop=True)
                    nc.tensor.matmul(
                        slot[:, 512:1024], kT[p][64:128, kcols],
                        qT[p][64:128, qcols], start=True, stop=True)
                    et = eps.tile([128, 1024], BF16, name="et")
                    nc.scalar.activation(
                        et[:], slot[:], mybir.ActivationFunctionType.Exp)
                    ets[kc] = et

                qk(0)
                if pend_norm[0] is not None:
                    pend_norm[0]()
                av_t = avps.tile([128, 1024], F32, name="av")

                def av(kc):
                    et = ets.pop(kc)
                    nc.tensor.matmul(
                        av_t[0:65, 0:512], v_aug[:, kc, 2 * p, :],
                        et[:, 0:512],
                        start=(kc == 0), stop=(kc == NKC - 1))
                    nc.tensor.matmul(
                        av_t[0:65, 512:1024], v_aug[:, kc, 2 * p + 1, :],
                        et[:, 512:1024],
                        start=(kc == 0), stop=(kc == NKC - 1))

                qk(1)
                if fillers:
                    fillers[0]()
                for kc in range(2, NKC):
                    qk(kc)
                    av(kc - 2)
                if len(fillers) > 1:
                    fillers[1]()
                av(NKC - 2)
                av(NKC - 1)
                lnd = rbs.tile([128, 1024], F32, name="lnd")
                nc.scalar.activation(
                    lnd[64:65, 0:1024], av_t[64:65, 0:1024],
                    mybir.ActivationFunctionType.Ln)
                recip = rbs.tile([128, 1024], BF16, name="recip")
                nc.scalar.activation(
                    recip[64:65, 0:1024], lnd[64:65, 0:1024],
                    mybir.ActivationFunctionType.Exp, scale=-1.0)
                pend_norm[0] = make_norm(p, qb, av_t, recip)

            # ---- prefix: v-proj (all kc) + q/k-proj for pair 0 ----
            for kc in range(NKC):
                vproj_group(kc)
            for qb in range(NQB):
                proj_group("q", 0, qb)
                proj_group("k", 0, qb)

            # ---- attention steps with interleaved proj / o-proj ----
            for i in range(NP * NQB):
                p, qb = i // NQB, i % NQB
                if i <= 11:
                    pn, j = i // 4 + 1, i % 4
                    fillers = [
                        (lambda pn=pn, j=j: proj_group("q", pn, j)),
                        (lambda pn=pn, j=j: proj_group("k", pn, j)),
                    ]
                elif i == 12:
                    fillers = []
                else:
                    blk = i - 13
                    fillers = [
                        (lambda blk=blk: (oproj_sc(4 * blk),
                                          oproj_sc(4 * blk + 1))),
                        (lambda blk=blk: (oproj_sc(4 * blk + 2),
                                          oproj_sc(4 * blk + 3))),
                    ]
                attention_step(p, qb, fillers)

            # ---- tail: last norm + last o-proj block ----
            pend_norm[0]()
            for sc in range(12, 16):
                oproj_sc(sc)

    nc.compile()
    return nc


def kernel(x, w_qkv, b_qkv, w_o, b_o):
    global LAST_EXEC_NS
    if _nc_cache[0] is None:
        _nc_cache[0] = _build()
    nc = _nc_cache[0]

    xT_b = [np.ascontiguousarray(x[b].T).astype(NBF).reshape(NIC, 128, S)
            for b in range(B)]
    w = w_qkv.astype(np.float32)
    in_maps = []
    for c in range(N_CORES):
        b, g = c // 2, c % 2
        # reference packs qkv per head: head h -> cols [h*192, (h+1)*192),
        # q dims 0:64, k 64:128, v 128:192 within
        heads = np.arange(g * 8, g * 8 + 8)
        qs = (heads[:, None] * 192 + np.arange(64)).ravel()
        ks = (heads[:, None] * 192 + 64 + np.arange(64)).ravel()
        vs = (heads[:, None] * 192 + 128 + np.arange(64)).ravel()
        in_maps.append({
            "xT": xT_b[b],
            "wq": (w[:, qs] / 8.0).astype(NBF).reshape(NIC, 128, 512),
            "wk": w[:, ks].astype(NBF).reshape(NIC, 128, 512),
            "wv": w[:, vs].astype(NBF).reshape(NIC, 128, 512),
            "wo": w_o[g * 512:(g + 1) * 512, :].astype(NBF).reshape(
                NP, 128, 1024),
            "bq": np.ascontiguousarray(
                (b_qkv[qs].astype(np.float32) / 8.0).reshape(NP, 128).T),
            "bk": np.ascontiguousarray(
                b_qkv[ks].astype(np.float32).reshape(NP, 128).T),
        })

    res = run_bass_kernel_spmd(nc, in_maps, list(range(N_CORES)),
                               trace=TRACE)
    LAST_EXEC_NS = res.exec_time_ns

    # v-bias folds out of attention: softmax rows sum to 1, so
    # vals_h = p_h @ (x W_vh) + b_vh and the b_vh term contributes the
    # constant row (concat_h b_vh) @ w_o
    vs_full = (np.arange(H)[:, None] * 192 + 128 + np.arange(HD)).ravel()
    bvwo = b_qkv[vs_full].astype(np.float32) @ w_o.astype(np.float32)

    out = np.empty((B, S, E), np.float32)
    bias = b_o.astype(np.float32) + bvwo
    for b in range(B):
        p0 = np.asarray(res.results[2 * b]["out"],
                        np.float32).reshape(S, E)
        p1 = np.asarray(res.results[2 * b + 1]["out"],
                        np.float32).reshape(S, E)
        out[b] = p0 + p1 + bias
    return out



# revision 4
# speedup vs baseline: 1.0581x; 1.0581x over previous
"""MultiHeadAttention on 8 TRN2 NeuronCores.

Shapes (hardcoded): x [4, 2048, 1024], w_qkv [1024, 3072], b_qkv [3072],
w_o [1024, 1024], b_o [1024]; H=16 heads, head_dim=64, scale 1/8.

Sharding: core c -> batch c//2, head-group c%2 (8 heads each).
Each core computes its 8 heads' attention values and a partial o-proj
([2048, 1024] f32); host sums the two partials per batch, adds b_o and
the constant row bv @ w_o (v-bias folds out of attention since softmax
rows sum to 1).

V2 single-phase schedule (all matmuls bf16): projections and o-proj
matmuls are interleaved into the attention steps so the PE array never
idles; the softmax normalization chain of each (pair, query-block) step
is deferred into the next step so the PE does not stall on the DVE
reciprocal; bc_e/bc_o broadcast matmuls stack into one PSUM bank at
partition bases 0/64.
"""

import os
import sys
import types

sys.path.insert(0, "/opt/trn_rl_repo")

import numpy as np
import ml_dtypes
from contextlib import ExitStack

import concourse.bass as bass  # noqa: F401
import concourse.tile as tile
from concourse import bacc, mybir
from concourse.bass_utils import run_bass_kernel_spmd

BF16 = mybir.dt.bfloat16
F32 = mybir.dt.float32
NBF = ml_dtypes.bfloat16

N_CORES = 8
B, S, D, E = 4, 2048, 1024, 1024
H, HD = 16, 64
NP = 4    # head pairs per core
NQB = 4   # query blocks of 512
NKC = 16  # key/seq chunks of 128
NIC = 8   # input-dim chunks of 128

TRACE = os.environ.get("KERNEL_TRACE", "") == "1"
LAST_EXEC_NS = None

if TRACE:
    _hook = [None]
    _ah = types.ModuleType("antenv.axon_hooks")
    _ah.set_axon_ntff_profile_hook = lambda h: _hook.__setitem__(0, h)
    _ah.get_axon_ntff_profile_hook = lambda: _hook[0]
    sys.modules["antenv.axon_hooks"] = _ah
    import antenv
    antenv.axon_hooks = _ah
    from trn_agent_boot.trn_boot import _ntff_profile_via_ctypes
    _ah.set_axon_ntff_profile_hook(
        _ntff_profile_via_ctypes("/opt/axon/libaxon_pjrt.so"))

_nc_cache = [None]


def _build():
    nc = bacc.Bacc("TRN2", target_bir_lowering=False, debug=False,
                   num_devices=N_CORES)
    xT_ap = nc.dram_tensor("xT", [NIC, 128, S], BF16, kind="ExternalInput").ap()
    wq_ap = nc.dram_tensor("wq", [NIC, 128, 512], BF16, kind="ExternalInput").ap()
    wk_ap = nc.dram_tensor("wk", [NIC, 128, 512], BF16, kind="ExternalInput").ap()
    wv_ap = nc.dram_tensor("wv", [NIC, 128, 512], BF16, kind="ExternalInput").ap()
    wo_ap = nc.dram_tensor("wo", [NP, 128, 1024], BF16, kind="ExternalInput").ap()
    bq_ap = nc.dram_tensor("bq", [128, NP], F32, kind="ExternalInput").ap()
    bk_ap = nc.dram_tensor("bk", [128, NP], F32, kind="ExternalInput").ap()
    out_ap = nc.dram_tensor("out", [NKC, 128, 1024], F32,
                            kind="ExternalOutput").ap()

    with tile.TileContext(nc) as tc:
        with ExitStack() as ctx:
            sb = ctx.enter_context(tc.tile_pool(name="sb", bufs=1))
            xT_sb = sb.tile([128, NIC, S], BF16)
            wq_sb = sb.tile([128, NIC, 512], BF16)
            wk_sb = sb.tile([128, NIC, 512], BF16)
            wv_sb = sb.tile([128, NIC, 512], BF16)
            wo_sb = sb.tile([128, NP, 1024], BF16)
            bq_sb = sb.tile([128, NP], F32)
            bk_sb = sb.tile([128, NP], F32)
            v_aug = sb.tile([128, NKC, 8, 65], BF16)
            ones_col = sb.tile([128, 64], BF16)
            qT = [sb.tile([128, S], BF16, name=f"qT{p}") for p in range(NP)]
            kT = [sb.tile([128, S], BF16, name=f"kT{p}") for p in range(NP)]
            valsT = [sb.tile([128, S], BF16, name=f"valsT{p}")
                     for p in range(NP)]

            for ic in range(NIC):
                nc.sync.dma_start(out=xT_sb[:, ic, :], in_=xT_ap[ic])
                nc.sync.dma_start(out=wv_sb[:, ic, :], in_=wv_ap[ic])
            for ic in range(NIC):
                nc.sync.dma_start(out=wk_sb[:, ic, :], in_=wk_ap[ic])
                nc.sync.dma_start(out=wq_sb[:, ic, :], in_=wq_ap[ic])
            nc.sync.dma_start(out=bq_sb[:], in_=bq_ap[:])
            nc.sync.dma_start(out=bk_sb[:], in_=bk_ap[:])
            for p in range(NP):
                nc.sync.dma_start(out=wo_sb[:, p, :], in_=wo_ap[p])
            nc.gpsimd.memset(v_aug[:], 1.0)
            nc.gpsimd.memset(ones_col[:], 1.0)

            qkps = ctx.enter_context(
                tc.tile_pool(name="qkps", bufs=2, space="PSUM"))
            avps = ctx.enter_context(
                tc.tile_pool(name="avps", bufs=1, space="PSUM"))
            accs = ctx.enter_context(
                tc.tile_pool(name="accs", bufs=2, space="PSUM"))
            eps = ctx.enter_context(tc.tile_pool(name="eps", bufs=4))
            rbs = ctx.enter_context(tc.tile_pool(name="rbs", bufs=2))
            ost = ctx.enter_context(tc.tile_pool(name="ost", bufs=2))

            def vproj_group(kc):
                acc = accs.tile([128, 512], F32, name="acc")
                for ic in range(NIC):
                    nc.tensor.matmul(
                        acc[:], xT_sb[:, ic, kc * 128:(kc + 1) * 128],
                        wv_sb[:, ic, :],
                        start=(ic == 0), stop=(ic == NIC - 1))
                nc.vector.tensor_copy(v_aug[:, kc, :, 0:64], acc[:])

            def proj_group(which, p, qb):
                qcols = slice(qb * 512, (qb + 1) * 512)
                pcols = slice(p * 128, (p + 1) * 128)
                w_sb, b_sb, dst = ((wq_sb, bq_sb, qT) if which == "q"
                                   else (wk_sb, bk_sb, kT))
                acc = accs.tile([128, 512], F32, name="acc")
                for ic in range(NIC):
                    nc.tensor.matmul(
                        acc[:], w_sb[:, ic, pcols], xT_sb[:, ic, qcols],
                        start=(ic == 0), stop=(ic == NIC - 1))
                nc.vector.tensor_scalar_add(
                    dst[p][:, qcols], acc[:], b_sb[:, p:p + 1])

            def oproj_sc(sc):
                scols = slice(sc * 128, (sc + 1) * 128)
                stage = ost.tile([128, 1024], F32, name="ostage")
                for half in range(2):
                    hcols = slice(half * 512, (half + 1) * 512)
                    og = accs.tile([128, 512], F32, name="acc")
                    for p in range(NP):
                        nc.tensor.matmul(
                            og[:], valsT[p][:, scols], wo_sb[:, p, hcols],
                            start=(p == 0), stop=(p == NP - 1))
                    nc.vector.tensor_copy(stage[:, hcols], og[:])
                nc.sync.dma_start(out=out_ap[sc], in_=stage[:])

            pend_norm = [None]

            def make_norm(p, qb, av, recip):
                qcols = slice(qb * 512, (qb + 1) * 512)

                def norm():
                    bc = accs.tile([128, 512], F32, name="acc")
                    nc.tensor.matmul(
                        bc[0:64, :], ones_col[64:65, :],
                        recip[64:65, 0:512], start=True, stop=True)
                    nc.tensor.matmul(
                        bc[64:128, :], ones_col[64:65, :],
                        recip[64:65, 512:1024], start=True, stop=True)
                    bc_sb = rbs.tile([128, 1024], BF16, name="bcsb")
                    nc.vector.tensor_copy(bc_sb[0:64, 0:512], bc[0:64, :])
                    nc.vector.tensor_copy(bc_sb[0:64, 512:1024],
                                          bc[64:128, :])
                    nc.vector.tensor_mul(
                        valsT[p][0:64, qcols], av[0:64, 0:512],
                        bc_sb[0:64, 0:512])
                    nc.vector.tensor_mul(
                        valsT[p][64:128, qcols], av[0:64, 512:1024],
                        bc_sb[0:64, 512:1024])
                return norm

            def attention_step(p, qb, fillers):
                qcols = slice(qb * 512, (qb + 1) * 512)
                ets = {}

                def qk(kc):
                    kcols = slice(kc * 128, (kc + 1) * 128)
                    slot = qkps.tile([128, 1024], F32, name="qkslot")
                    nc.tensor.matmul(
                        slot[:, 0:512], kT[p][0:64, kcols],
                        qT[p][0:64, qcols], start=True, stop=True)
                    nc.tensor.matmul(
                        slot[:, 512:1024], kT[p][64:128, kcols],
                        qT[p][64:128, qcols], start=True, stop=True)
                    et = eps.tile([128, 1024], BF16, name="et")
                    nc.scalar.activation(
                        et[:], slot[:], mybir.ActivationFunctionType.Exp)
                    ets[kc] = et

                qk(0)
                if pend_norm[0] is not None:
                    pend_norm[0]()
                av_t = avps.tile([128, 1024], F32, name="av")

                def av(kc):
                    et = ets.pop(kc)
                    nc.tensor.matmul(
                        av_t[0:65, 0:512], v_aug[:, kc, 2 * p, :],
                        et[:, 0:512],
                        start=(kc == 0), stop=(kc == NKC - 1))
                    nc.tensor.matmul(
                        av_t[0:65, 512:1024], v_aug[:, kc, 2 * p + 1, :],
                        et[:, 512:1024],
                        start=(kc == 0), stop=(kc == NKC - 1))

                qk(1)
                if fillers:
                    fillers[0]()
                for kc in range(2, NKC):
                    qk(kc)
                    av(kc - 2)
                if len(fillers) > 1:
                    fillers[1]()
                av(NKC - 2)
                av(NKC - 1)
                # full-partition op: the custom DVE recip silently fails on
                # base_partition != 0; rows other than 64 are discarded
                recip_f = rbs.tile([128, 1024], F32, name="recipf")
                nc.vector.reciprocal_approx_fast(
                    recip_f[:, 0:1024], av_t[:, 0:1024])
                recip = rbs.tile([128, 1024], BF16, name="recip")
                nc.vector.tensor_copy(
                    recip[64:65, 0:1024], recip_f[64:65, 0:1024])
                pend_norm[0] = make_norm(p, qb, av_t, recip)

            # ---- prefix: v-proj (all kc) + q/k-proj for pair 0 ----
            for kc in range(NKC):
                vproj_group(kc)
            for qb in range(NQB):
                proj_group("q", 0, qb)
                proj_group("k", 0, qb)

            # ---- attention steps with interleaved proj / o-proj ----
            for i in range(NP * NQB):
                p, qb = i // NQB, i % NQB
                if i <= 11:
                    pn, j = i // 4 + 1, i % 4
                    fillers = [
                        (lambda pn=pn, j=j: proj_group("q", pn, j)),
                        (lambda pn=pn, j=j: proj_group("k", pn, j)),
                    ]
                elif i == 12:
                    fillers = []
                else:
                    blk = i - 13
                    fillers = [
                        (lambda blk=blk: (oproj_sc(4 * blk),
                                          oproj_sc(4 * blk + 1))),
                        (lambda blk=blk: (oproj_sc(4 * blk + 2),
                                          oproj_sc(4 * blk + 3))),
                    ]
                attention_step(p, qb, fillers)

            # ---- tail: last norm + last o-proj block ----
            pend_norm[0]()
            for sc in range(12, 16):
                oproj_sc(sc)

    nc.compile()
    return nc


def kernel(x, w_qkv, b_qkv, w_o, b_o):
    global LAST_EXEC_NS
    if _nc_cache[0] is None:
        _nc_cache[0] = _build()
    nc = _nc_cache[0]

    xT_b = [np.ascontiguousarray(x[b].T).astype(NBF).reshape(NIC, 128, S)
            for b in range(B)]
    w = w_qkv.astype(np.float32)
    in_maps = []
    for c in range(N_CORES):
        b, g = c // 2, c % 2
        # reference packs qkv per head: head h -> cols [h*192, (h+1)*192),
        # q dims 0:64, k 64:128, v 128:192 within
        heads = np.arange(g * 8, g * 8 + 8)
        qs = (heads[:, None] * 192 + np.arange(64)).ravel()
        ks = (heads[:, None] * 192 + 64 + np.arange(64)).ravel()
        vs = (heads[:, None] * 192 + 128 + np.arange(64)).ravel()
        in_maps.append({
            "xT": xT_b[b],
            "wq": (w[:, qs] / 8.0).astype(NBF).reshape(NIC, 128, 512),
            "wk": w[:, ks].astype(NBF).reshape(NIC, 128, 512),
            "wv": w[:, vs].astype(NBF).reshape(NIC, 128, 512),
            "wo": w_o[g * 512:(g + 1) * 512, :].astype(NBF).reshape(
                NP, 128, 1024),
            "bq": np.ascontiguousarray(
                (b_qkv[qs].astype(np.float32) / 8.0).reshape(NP, 128).T),
            "bk": np.ascontiguousarray(
                b_qkv[ks].astype(np.float32).reshape(NP, 128).T),
        })

    res = run_bass_kernel_spmd(nc, in_maps, list(range(N_CORES)),
                               trace=TRACE)
    LAST_EXEC_NS = res.exec_time_ns

    # v-bias folds out of attention: softmax rows sum to 1, so
    # vals_h = p_h @ (x W_vh) + b_vh and the b_vh term contributes the
    # constant row (concat_h b_vh) @ w_o
    vs_full = (np.arange(H)[:, None] * 192 + 128 + np.arange(HD)).ravel()
    bvwo = b_qkv[vs_full].astype(np.float32) @ w_o.astype(np.float32)

    out = np.empty((B, S, E), np.float32)
    bias = b_o.astype(np.float32) + bvwo
    for b in range(B):
        p0 = np.asarray(res.results[2 * b]["out"],
                        np.float32).reshape(S, E)
        p1 = np.asarray(res.results[2 * b + 1]["out"],
                        np.float32).reshape(S, E)
        out[b] = p0 + p1 + bias
    return out



# revision 36
# speedup vs baseline: 1.2339x; 1.1661x over previous
"""MultiHeadAttention on 8 TRN2 NeuronCores.

Shapes (hardcoded): x [4, 2048, 1024], w_qkv [1024, 3072], b_qkv [3072],
w_o [1024, 1024], b_o [1024]; H=16 heads, head_dim=64, scale 1/8.

Sharding: core c -> batch c//2, head-group c%2 (8 heads each).
Each core computes its 8 heads' attention values and a partial o-proj
([2048, 1024] f32); host sums the two partials per batch, adds b_o and
the constant row bv @ w_o (v-bias folds out of attention since softmax
rows sum to 1).

V3 schedule: single-phase (projections/o-proj interleaved into the
attention steps), with
 - q/k/v projections in fp8e4 DoubleRow (K=256 per matmul). Weights are
   scaled x32 on the host so they sit in fp8's normal range; q/k scaling
   cancels via the exp scale (1/(32*32*8)), v scaling via the broadcast
   constant (1/32 instead of 1).
 - av matmuls in fp8e4 DoubleRow over key-chunk pairs (K=256); exp
   writes et directly as fp8 (values ~e^{N(0,1/9)} sit in fp8's normal
   range).
 - softmax reciprocal on DVE (reciprocal_approx_fast, full-partition:
   the custom op silently fails on base_partition != 0) instead of the
   scalar engine's ln/exp chain; keeps ACT free for the 256 exps and
   the step boundary stall-free (HAM stays warm).
 - deferred norm: each step's normalize chain runs early in the NEXT
   step; av issue slots are placed late enough to not block the
   in-order PE queue on the avps release.
"""

import os
import sys
import types

sys.path.insert(0, "/opt/trn_rl_repo")

import numpy as np
import ml_dtypes
from contextlib import ExitStack

import concourse.bass as bass  # noqa: F401
import concourse.tile as tile
from concourse import bacc, mybir
from concourse.bass_utils import run_bass_kernel_spmd

BF16 = mybir.dt.bfloat16
F32 = mybir.dt.float32
F8 = mybir.dt.float8e4
NBF = ml_dtypes.bfloat16
NF8 = ml_dtypes.float8_e4m3
DR = mybir.MatmulPerfMode.DoubleRow

N_CORES = 8
B, S, D, E = 4, 2048, 1024, 1024
H, HD = 16, 64
NP = 4    # head pairs per core
NQB = 4   # query blocks of 512
NKC = 16  # key/seq chunks of 128
NIC = 8   # input-dim chunks of 128

WS = 32.0           # host-side weight scale into fp8 normal range
EXP_SCALE = 1.0 / (WS * WS * 8.0)
AV_FP8 = os.environ.get("AV_FP8", "0") == "1"
PROJ_DR = os.environ.get("PROJ_DR", "0") == "1"

TRACE = os.environ.get("KERNEL_TRACE", "") == "1"
LAST_EXEC_NS = None

if TRACE:
    _hook = [None]
    _ah = types.ModuleType("antenv.axon_hooks")
    _ah.set_axon_ntff_profile_hook = lambda h: _hook.__setitem__(0, h)
    _ah.get_axon_ntff_profile_hook = lambda: _hook[0]
    sys.modules["antenv.axon_hooks"] = _ah
    import antenv
    antenv.axon_hooks = _ah
    from trn_agent_boot.trn_boot import _ntff_profile_via_ctypes
    _ah.set_axon_ntff_profile_hook(
        _ntff_profile_via_ctypes("/opt/axon/libaxon_pjrt.so"))

_nc_cache = [None]


def _build():
    nc = bacc.Bacc("TRN2", target_bir_lowering=False, debug=False,
                   num_devices=N_CORES)
    XDT = F8 if PROJ_DR else BF16
    xT_ap = nc.dram_tensor("xT", [NIC, 128, S], XDT, kind="ExternalInput").ap()
    wq_ap = nc.dram_tensor("wq", [NIC, 128, 512], XDT, kind="ExternalInput").ap()
    wk_ap = nc.dram_tensor("wk", [NIC, 128, 512], XDT, kind="ExternalInput").ap()
    wv_ap = nc.dram_tensor("wv", [NIC, 128, 512], XDT, kind="ExternalInput").ap()
    wo_ap = nc.dram_tensor("wo", [NP, 128, 1024], BF16, kind="ExternalInput").ap()
    bq_ap = nc.dram_tensor("bq", [128, NP], F32, kind="ExternalInput").ap()
    bk_ap = nc.dram_tensor("bk", [128, NP], F32, kind="ExternalInput").ap()
    out_ap = nc.dram_tensor("out", [NKC, 128, 1024], F32,
                            kind="ExternalOutput").ap()

    with tile.TileContext(nc) as tc:
        with ExitStack() as ctx:
            sb = ctx.enter_context(tc.tile_pool(name="sb", bufs=1))
            xT_sb = sb.tile([128, NIC, S], XDT)
            wq_sb = sb.tile([128, NIC, 512], XDT)
            wk_sb = sb.tile([128, NIC, 512], XDT)
            wv_sb = sb.tile([128, NIC, 512], XDT)
            wo_sb = sb.tile([128, NP, 1024], BF16)
            bq_sb = sb.tile([128, NP], F32)
            bk_sb = sb.tile([128, NP], F32)
            # v (x32 scale) per (kc-pair, head, kc-parity); 80-col pad keeps
            # the DoubleRow Ko stride 16B-aligned; col 64 stays 1.0 for the
            # softmax denominator row.
            if AV_FP8:
                v_aug = sb.tile([128, NKC // 2, 8, 2, 80], F8)
            else:
                v_aug = sb.tile([128, NKC, 8, 65], BF16)
            ones_col = sb.tile([128, 128], BF16)
            # zero-padded q: [:, 0, :] holds head A on partitions 0:64
            # (zeros below), [:, 1, :] holds head B on partitions 64:128
            # (zeros above). Lets qk run as ONE K=128 N=1024 matmul in the
            # default 128x128 PE mode — the auto-inferred 64-row tiling of
            # the two K=64 matmuls forced a PE mode-switch drain against
            # the 128x128 av/proj matmuls at every transition.
            # layout [128, qb, 2, 512]: the two padded halves are contiguous
            # per query block so the qk moving AP merges to 2D [128, 1024]
            qT = [sb.tile([128, NQB, 2, 512], BF16, name=f"qT{p}")
                  for p in range(NP)]
            kT = [sb.tile([128, S], BF16, name=f"kT{p}") for p in range(NP)]
            valsT = [sb.tile([128, S], BF16, name=f"valsT{p}")
                     for p in range(NP)]
            # zero-padded reciprocal rows (only row 64 ever written) so the
            # bc broadcast matmuls run full-K at tile (0,0); manual
            # double-buffer by step parity
            recip_pad = [sb.tile([128, 1024], BF16, name=f"rpad{j}")
                         for j in range(2)]

            for ic in range(NIC):
                nc.sync.dma_start(out=xT_sb[:, ic, :], in_=xT_ap[ic])
                nc.sync.dma_start(out=wv_sb[:, ic, :], in_=wv_ap[ic])
            for ic in range(NIC):
                nc.sync.dma_start(out=wk_sb[:, ic, :], in_=wk_ap[ic])
                nc.sync.dma_start(out=wq_sb[:, ic, :], in_=wq_ap[ic])
            nc.sync.dma_start(out=bq_sb[:], in_=bq_ap[:])
            nc.sync.dma_start(out=bk_sb[:], in_=bk_ap[:])
            for p in range(NP):
                nc.sync.dma_start(out=wo_sb[:, p, :], in_=wo_ap[p])
            nc.gpsimd.memset(v_aug[:], 1.0)
            # bc matmul constant: folds the 1/32 v unscale into the
            # denominator broadcast
            nc.gpsimd.memset(ones_col[:], 1.0 / WS)
            for j in range(2):
                nc.gpsimd.memset(recip_pad[j][:], 0.0)
                nc.gpsimd.memset(qT[j][:], 0.0)
                nc.gpsimd.memset(qT[2 + j][:], 0.0)

            qkps = ctx.enter_context(
                tc.tile_pool(name="qkps", bufs=2, space="PSUM"))
            avps = ctx.enter_context(
                tc.tile_pool(name="avps", bufs=1, space="PSUM"))
            accs = ctx.enter_context(
                tc.tile_pool(name="accs", bufs=2, space="PSUM"))
            eps = ctx.enter_context(tc.tile_pool(name="eps", bufs=4))
            rbs = ctx.enter_context(tc.tile_pool(name="rbs", bufs=2))
            ost = ctx.enter_context(tc.tile_pool(name="ost", bufs=2))

            def vproj_group(kc):
                acc = accs.tile([128, 512], F32, name="acc")
                if PROJ_DR:
                    for ic in range(0, NIC, 2):
                        nc.tensor.matmul(
                            acc[:],
                            xT_sb[:, ic:ic + 2, kc * 128:(kc + 1) * 128],
                            wv_sb[:, ic:ic + 2, :],
                            start=(ic == 0), stop=(ic == NIC - 2),
                            perf_mode=DR)
                else:
                    for ic in range(NIC):
                        nc.tensor.matmul(
                            acc[:], xT_sb[:, ic, kc * 128:(kc + 1) * 128],
                            wv_sb[:, ic, :],
                            start=(ic == 0), stop=(ic == NIC - 1))
                if AV_FP8:
                    nc.vector.tensor_copy(
                        v_aug[:, kc // 2, :, kc % 2, 0:64], acc[:])
                else:
                    nc.vector.tensor_copy(v_aug[:, kc, :, 0:64], acc[:])

            def proj_group(which, p, qb):
                qcols = slice(qb * 512, (qb + 1) * 512)
                pcols = slice(p * 128, (p + 1) * 128)
                w_sb, b_sb, dst = ((wq_sb, bq_sb, qT) if which == "q"
                                   else (wk_sb, bk_sb, kT))
                acc = accs.tile([128, 512], F32, name="acc")
                if PROJ_DR:
                    for ic in range(0, NIC, 2):
                        nc.tensor.matmul(
                            acc[:], w_sb[:, ic:ic + 2, pcols],
                            xT_sb[:, ic:ic + 2, qcols],
                            start=(ic == 0), stop=(ic == NIC - 2),
                            perf_mode=DR)
                else:
                    for ic in range(NIC):
                        nc.tensor.matmul(
                            acc[:], w_sb[:, ic, pcols], xT_sb[:, ic, qcols],
                            start=(ic == 0), stop=(ic == NIC - 1))
                if which == "q":
                    nc.vector.tensor_scalar_add(
                        dst[p][0:64, qb, 0, :], acc[0:64, :],
                        b_sb[0:64, p:p + 1])
                    nc.vector.tensor_scalar_add(
                        dst[p][64:128, qb, 1, :], acc[64:128, :],
                        b_sb[64:128, p:p + 1])
                else:
                    nc.vector.tensor_scalar_add(
                        dst[p][:, qcols], acc[:], b_sb[:, p:p + 1])

            def oproj_sc(sc):
                scols = slice(sc * 128, (sc + 1) * 128)
                stage = ost.tile([128, 1024], F32, name="ostage")
                for half in range(2):
                    hcols = slice(half * 512, (half + 1) * 512)
                    og = accs.tile([128, 512], F32, name="acc")
                    for p in range(NP):
                        nc.tensor.matmul(
                            og[:], valsT[p][:, scols], wo_sb[:, p, hcols],
                            start=(p == 0), stop=(p == NP - 1))
                    nc.vector.tensor_copy(stage[:, hcols], og[:])
                nc.sync.dma_start(out=out_ap[sc], in_=stage[:])

            pend_norm = [None]

            def make_norm(p, qb, av, recip):
                qcols = slice(qb * 512, (qb + 1) * 512)

                def norm():
                    # recip is zero-padded except row 64, so full-K matmuls
                    # at tile (0,0) compute the broadcast without a PE
                    # mode switch; ones_col carries the 1/WS unscale
                    bc = accs.tile([128, 512], F32, name="acc")
                    nc.tensor.matmul(
                        bc[:, :], ones_col[:, :],
                        recip[:, 0:512], start=True, stop=True)
                    bc2 = accs.tile([128, 512], F32, name="acc")
                    nc.tensor.matmul(
                        bc2[:, :], ones_col[:, :],
                        recip[:, 512:1024], start=True, stop=True)
                    bc_sb = rbs.tile([128, 1024], BF16, name="bcsb")
                    nc.vector.tensor_copy(bc_sb[0:64, 0:512], bc[0:64, :])
                    nc.vector.tensor_copy(bc_sb[0:64, 512:1024],
                                          bc2[0:64, :])
                    nc.vector.tensor_mul(
                        valsT[p][0:64, qcols], av[0:64, 0:512],
                        bc_sb[0:64, 0:512])
                    nc.vector.tensor_mul(
                        valsT[p][64:128, qcols], av[0:64, 512:1024],
                        bc_sb[0:64, 512:1024])
                return norm

            def attention_step(p, qb, fillers, norm_first=False, si=0):
                qcols = slice(qb * 512, (qb + 1) * 512)
                etps = {}

                def qk(kc):
                    kcols = slice(kc * 128, (kc + 1) * 128)
                    slot = qkps.tile([128, 1024], F32, name="qkslot")
                    # two K=128 N=512 matmuls (PSUM bank limit), both full
                    # mode at tile (0,0) with the same kT stationary: the
                    # zero halves of qT kill the cross-head terms
                    nc.tensor.matmul(
                        slot[:, 0:512], kT[p][:, kcols],
                        qT[p][:, qb, 0, :], start=True, stop=True)
                    nc.tensor.matmul(
                        slot[:, 512:1024], kT[p][:, kcols],
                        qT[p][:, qb, 1, :], start=True, stop=True)
                    if AV_FP8:
                        if kc % 2 == 0:
                            etps[kc // 2] = eps.tile([128, 2, 1024], F8,
                                                     name="etp")
                        dst = etps[kc // 2][:, kc % 2, :]
                    else:
                        etps[kc] = eps.tile([128, 1024], BF16, name="etp")
                        dst = etps[kc][:, :]
                    nc.scalar.activation(
                        dst, slot[:],
                        mybir.ActivationFunctionType.Exp, scale=EXP_SCALE)

                av_t = avps.tile([128, 1024], F32, name="av")

                def av(j):
                    if AV_FP8:
                        etp = etps.pop(j)
                        for h in range(2):
                            hc = slice(h * 512, (h + 1) * 512)
                            nc.tensor.matmul(
                                av_t[0:65, hc],
                                v_aug[:, j, 2 * p + h, :, 0:65],
                                etp[:, :, hc],
                                start=(j == 0), stop=(j == NKC // 2 - 1),
                                perf_mode=DR)
                    else:
                        for kc in (2 * j, 2 * j + 1):
                            et = etps[kc] if kc < NKC - 1 else etps.pop(kc)
                            for h in range(2):
                                hc = slice(h * 512, (h + 1) * 512)
                                nc.tensor.matmul(
                                    av_t[0:65, hc],
                                    v_aug[:, kc, 2 * p + h, :],
                                    et[:, hc],
                                    start=(kc == 0), stop=(kc == NKC - 1))

                # issue slots: av(j) late enough that (a) exp(2j+1) is done,
                # (b) av(0) does not block the in-order PE queue on the
                # previous step's norm muls (avps release); norm at qk(4) so
                # its bc matmuls don't stall the queue on the DVE recip;
                # fillers early/late where PE has slack.
                av_after = {6: 0, 7: 1, 8: 2, 9: 3, 10: 4, 12: 5, 14: 6}
                qk(0)
                qk(1)
                if norm_first:
                    # o-proj fillers read valsT written by the previous
                    # step's deferred norm — it must be issued first
                    if pend_norm[0] is not None:
                        pend_norm[0]()
                    qk(2)
                    if fillers:
                        fillers[0]()
                    qk(3)
                    qk(4)
                else:
                    qk(2)
                    if fillers:
                        fillers[0]()
                    qk(3)
                    qk(4)
                    if pend_norm[0] is not None:
                        pend_norm[0]()
                for kc in range(5, NKC):
                    qk(kc)
                    if kc in av_after:
                        av(av_after[kc])
                    if kc == 11 and len(fillers) > 1:
                        # PE slack slot; keeping it out of the step tail so
                        # it does not delay av(7) -> next step's qk/exp
                        fillers[1]()
                av(NKC // 2 - 1)
                # full-partition op: the custom DVE recip silently fails on
                # base_partition != 0; rows other than 64 are discarded
                recip_f = rbs.tile([128, 1024], F32, name="recipf")
                nc.vector.reciprocal_approx_fast(
                    recip_f[:, 0:1024], av_t[:, 0:1024])
                recip = recip_pad[si % 2]
                nc.vector.tensor_copy(
                    recip[64:65, 0:1024], recip_f[64:65, 0:1024])
                pend_norm[0] = make_norm(p, qb, av_t, recip)

            # ---- prefix: v-proj (all kc) + q/k-proj for pair 0 ----
            for kc in range(NKC):
                vproj_group(kc)
            for qb in range(NQB):
                proj_group("q", 0, qb)
                proj_group("k", 0, qb)

            # ---- attention steps with interleaved proj / o-proj ----
            for i in range(NP * NQB):
                p, qb = i // NQB, i % NQB
                if i <= 11:
                    pn, j = i // 4 + 1, i % 4
                    fillers = [
                        (lambda pn=pn, j=j: proj_group("q", pn, j)),
                        (lambda pn=pn, j=j: proj_group("k", pn, j)),
                    ]
                elif i == 12:
                    fillers = []
                else:
                    blk = i - 13
                    fillers = [
                        (lambda blk=blk: (oproj_sc(4 * blk),
                                          oproj_sc(4 * blk + 1))),
                        (lambda blk=blk: (oproj_sc(4 * blk + 2),
                                          oproj_sc(4 * blk + 3))),
                    ]
                attention_step(p, qb, fillers, norm_first=(i >= 13), si=i)

            # ---- tail: last norm + last o-proj block ----
            pend_norm[0]()
            for sc in range(12, 16):
                oproj_sc(sc)

    nc.compile()
    return nc


def kernel(x, w_qkv, b_qkv, w_o, b_o):
    global LAST_EXEC_NS
    if _nc_cache[0] is None:
        _nc_cache[0] = _build()
    nc = _nc_cache[0]

    NXDT = NF8 if PROJ_DR else NBF
    xT_b = [np.ascontiguousarray(x[b].T).astype(NXDT).reshape(NIC, 128, S)
            for b in range(B)]
    w = w_qkv.astype(np.float32) * WS
    bqkv = b_qkv.astype(np.float32) * WS
    in_maps = []
    for c in range(N_CORES):
        b, g = c // 2, c % 2
        # reference packs qkv per head: head h -> cols [h*192, (h+1)*192),
        # q dims 0:64, k 64:128, v 128:192 within
        heads = np.arange(g * 8, g * 8 + 8)
        qs = (heads[:, None] * 192 + np.arange(64)).ravel()
        ks = (heads[:, None] * 192 + 64 + np.arange(64)).ravel()
        vs = (heads[:, None] * 192 + 128 + np.arange(64)).ravel()
        in_maps.append({
            "xT": xT_b[b],
            "wq": w[:, qs].astype(NXDT).reshape(NIC, 128, 512),
            "wk": w[:, ks].astype(NXDT).reshape(NIC, 128, 512),
            "wv": w[:, vs].astype(NXDT).reshape(NIC, 128, 512),
            "wo": w_o[g * 512:(g + 1) * 512, :].astype(NBF).reshape(
                NP, 128, 1024),
            "bq": np.ascontiguousarray(bqkv[qs].reshape(NP, 128).T),
            "bk": np.ascontiguousarray(bqkv[ks].reshape(NP, 128).T),
        })

    res = run_bass_kernel_spmd(nc, in_maps, list(range(N_CORES)),
                               trace=TRACE)
    LAST_EXEC_NS = res.exec_time_ns

    # v-bias folds out of attention: softmax rows sum to 1, so
    # vals_h = p_h @ (x W_vh) + b_vh and the b_vh term contributes the
    # constant row (concat_h b_vh) @ w_o
    vs_full = (np.arange(H)[:, None] * 192 + 128 + np.arange(HD)).ravel()
    bvwo = b_qkv[vs_full].astype(np.float32) @ w_o.astype(np.float32)

    out = np.empty((B, S, E), np.float32)
    bias = b_o.astype(np.float32) + bvwo
    for b in range(B):
        p0 = np.asarray(res.results[2 * b]["out"],
                        np.float32).reshape(S, E)
        p1 = np.asarray(res.results[2 * b + 1]["out"],
                        np.float32).reshape(S, E)
        out[b] = p0 + p1 + bias
    return out
